# revision 19
# baseline (speedup 1.0000x reference)
"""Trainium2 Bass kernel for 2D block-local multi-head attention (v3).

Problem (hardcoded): x [1,128,48,64] -> 3x3 conv projections to q/k/v
(d_model=32, 8 heads, d_head=4), t2t local_attention_2d with
query_shape=(128,24), memory_flange=(8,8), combine heads, 3x3 output conv.

Structural facts (see reference): H=128, W=48 -> 2 query blocks (128x24);
the flange is all zero padding, so block b attends the static 128x32 strip
of real pixels: queries cols [24b,24b+24), keys cols [16b,16b+32).

Sharding: one head per NeuronCore (8 heads / 8 cores), no cross-core
communication. Each core computes its head's q/k/v conv, block-local
attention, and a partial output conv over its 4 channels; host sums the
8 partial [64, 6144] results.

Design - the original baseline was ACT-bound (exp of 25.2M logits/core at
0.83 ns/row ~= 200us). Structural changes:
  * exp split across TWO engines: ACT computes exact Exp for ~17/32 key
    tiles per granule; DVE computes the rest with a one-instruction
    Schraudolph exp: int16 = round(logit * 128/ln2 + (127*128 + boff)),
    bitcast to bf16. The approximation error is a smooth function of the
    logit which softmax normalization largely cancels (measured
    end-to-end rel err 5.0e-3 vs 3.3e-3 for the bf16 baseline).
  * AV uses exp-stationary matmuls: out[128q, 8] = ex_tile[128k,128q]^T
    @ V'_kt[128k, 8] accumulated over 32 key tiles into per-qtile psum
    accumulators sharing one psum bank (single start/stop group).
    V' holds v in cols 0:4 and 1.0 in cols 4:8 (softmax denominator).
  * the attention is a single flattened stream of (block, granule,
    key-tile) units: logits(u_i) || exp(u_i) || AV(u_{i-4}), with the
    granule epilogue (normalize on DVE/Pool, PE transpose to o^T)
    emitted a few units late so no engine ever blocks on it.
  * granule (0,0)'s first 16 key tiles only need image rows 0:64 of
    k/q, so they interleave with conv chunks 8..15 - the exp engines
    start ~15us earlier (their AVs are flushed once V' is built).
  * qb/kb/vb block repacks are strided sbuf->sbuf DMAs, per half-image.
  * output conv is a single 36-deep contraction (9 taps x 4 ch) per row
    chunk reading a 36-partition oo buffer whose row-blocks are
    tap-shifted copies of o^T, DMA-scattered in two row-waves per block;
    chunks 0..7 run during the last granule, the rest at the end.
  * PSUM banks (8): lgp 3x2 + cps 2 (conv, freed) -> vps 1 (V' build,
    freed) -> ops 1 + avs 1 + ops2 1. DMA issue spread over SP/ACT
    HWDGE queues + Pool SWDGE (HWDGE serializes ~0.65us per DMA).
"""

import contextlib

import numpy as np

H, W, CIN, DM, NH, DH = 128, 48, 64, 32, 8, 4
HP, WP = H + 2, W + 2          # padded spatial dims for 3x3 SAME conv
PADN = HP * WP + 4             # padded flat buffer size (+4 tail slack)
NPIX = H * W                   # 6144
QW, KW = 24, 32                # per-block query/key column widths
NQ = H * QW                    # 3072 queries per block
NK = H * KW                    # 4096 keys per block
NKT = 32                       # key tiles (128 keys each) per block
G = 1024                       # query granule (psum tile width)
NG = NQ // G                   # 3 granules per block
NQT = G // 128                 # 8 q-subtiles per granule
CHUNK_ROWS = 8                 # conv output rows per matmul chunk
NCHUNK = H // CHUNK_ROWS       # 16
CN = CHUNK_ROWS * WP           # conv matmul free size, 400
GUARD = 64                     # left guard in oo (negative tap shifts)
OO_N = GUARD + HP * WP + 8     # oo depth per partition (bf16 elems)
A_EXP = float((1 << 7) / np.log(2.0))    # 184.665 = 2^7 * log2(e)
B_EXP = float((127 << 7) - 5.5)          # exponent bias + mantissa tuning
ACT_TILES = 17                 # per-granule ACT-exp share (of 32)
AV_LAG = 4                     # AV trails logits by this many stream units
# granule processing order: both blocks' granule 0 first (their first 16
# key tiles can interleave with the conv), then the rest; block scatter
# waves need g0+g1 (rows 0:80) resp. g2 (rows 80:128) of a block done.
SEQ = [(0, 0), (1, 0), (0, 1), (1, 1), (0, 2), (1, 2)]

_cached = {}


def _act_set(nact):
    # interleave ACT/DVE assignments so both engines start immediately
    s = set(range(0, 2 * min(nact, 16), 2))
    extra = nact - len(s)
    odds = list(range(31, 0, -2))
    return s | set(odds[:extra])


def _build_nc():
    import concourse.bacc as bacc
    import concourse.tile as tile
    import concourse.mybir as mybir

    f32 = mybir.dt.float32
    bf16 = mybir.dt.bfloat16
    i16 = mybir.dt.int16

    nc = bacc.Bacc("TRN2", target_bir_lowering=False)

    xx_d = nc.dram_tensor("xx", [128, PADN], bf16, kind="ExternalInput")
    wqkv_d = nc.dram_tensor("wqkv", [128, 6 * 12], bf16, kind="ExternalInput")
    bias_d = nc.dram_tensor("bias12", [12, 1], f32, kind="ExternalInput")
    wo36_d = nc.dram_tensor("wo36", [36, 64], bf16, kind="ExternalInput")
    id4_d = nc.dram_tensor("id4", [DH, DH], bf16, kind="ExternalInput")
    id128_d = nc.dram_tensor("id128", [128, 128], bf16, kind="ExternalInput")
    outp_d = nc.dram_tensor("outp", [CIN, NPIX], f32, kind="ExternalOutput")

    ACT_SET = {(kt, p) for p in range(2)
               for kt in _act_set(ACT_TILES + p)}
    dma_rr = [0]

    with tile.TileContext(nc) as tc:
        stack = contextlib.ExitStack()

        def dma(dst, src, pool_ok=True):
            # spread DMA issue: SP and (startup only) ACT hwdge queues plus
            # the Pool swdge queue; ACT's queue is avoided mid-attention
            # since a DMA holds its SEQ ~1us, stalling the exp stream.
            engs = (nc.sync, nc.gpsimd) if pool_ok else (nc.sync, nc.scalar)
            eng = engs[dma_rr[0] % 2]
            dma_rr[0] += 1
            return eng.dma_start(dst, src)

        mp = stack.enter_context(tc.tile_pool(name="main", bufs=1))
        if True:
            xx = mp.tile([128, PADN], bf16)
            wqkv = mp.tile([128, 6 * 12], bf16)
            bias12 = mp.tile([12, 1], f32)
            wo36 = mp.tile([36, 64], bf16)
            id4 = mp.tile([DH, DH], bf16)
            id128 = mp.tile([128, 128], bf16)
            qkvT = mp.tile([12, NPIX], bf16)
            qb = mp.tile([DH, 2 * NQ], bf16)
            kb = mp.tile([DH, 2 * NK], bf16)
            vTb = mp.tile([DH, 2 * NK], bf16)
            vp = mp.tile([128, 2 * NKT * 8], bf16)   # V' tiles, ones in 4:8
            oT = mp.tile([DH, 2 * NQ], bf16)         # normalized o^T
            oo = mp.tile([36, OO_N], bf16)           # 9 tap-shifted o^T
            zbias = mp.tile([128, 1], f32)
            rec = mp.tile([128, NQT], f32)
            av_sb = mp.tile([128, NQT * 8], f32)
            o_sb = mp.tile([128, NQT * DH], bf16)
            actwarm = mp.tile([128, 1], f32)
            pewarm = mp.tile([DH, 512], bf16)

            nc.sync.dma_start(wqkv[:], wqkv_d.ap())
            nc.sync.dma_start(bias12[:], bias_d.ap())
            nc.scalar.dma_start(wo36[:], wo36_d.ap())
            nc.scalar.dma_start(id4[:], id4_d.ap())
            nc.scalar.dma_start(id128[:], id128_d.ap())
            xx_ap = xx_d.ap()
            xx_cuts = (0, 512, 1536, 3072, 4800, PADN)
            for q4 in range(5):
                s4, e4 = xx_cuts[q4], xx_cuts[q4 + 1]
                dma(xx[:, s4:e4], xx_ap[:, s4:e4], pool_ok=False)

            # init memsets on otherwise-idle engines
            nc.gpsimd.memset(vp[:], 1.0)
            nc.gpsimd.memset(oo[:], 0.0)
            nc.vector.memset(zbias[:], 0.0)
            nc.vector.memset(pewarm[:], 1.0)
            # preload the ACT exp table off the critical path
            nc.scalar.activation(
                actwarm[:], zbias[:], mybir.ActivationFunctionType.Exp,
                bias=zbias[:],
            )
            # PE clock warmup: dummy matmuls on uninitialized SBUF
            with tc.tile_pool(name="wps", bufs=1, space="PSUM") as wps:
                wp = wps.tile([DH, 512], f32, tag="wp")
                for _ in range(6):
                    nc.tensor.matmul(wp[:], pewarm[:, 0:DH], pewarm[:],
                                     start=True, stop=True)

            qkvT_v = qkvT[:].rearrange("p (h w) -> p h w", w=W)
            vp_v = vp[:].rearrange("p (t e) -> p t e", e=8)

            lgp = stack.enter_context(
                tc.tile_pool(name="lgp", bufs=3, space="PSUM"))
            exp_pool = stack.enter_context(tc.tile_pool(name="exp", bufs=36))
            ost = stack.enter_context(tc.tile_pool(name="ost", bufs=2))

            # ---- attention stream state (emission-order based) ----
            st = {"avbig": None, "avt": {}, "exs": {}, "elist": [], "navd": 0,
                  "avn": {}, "epi2q": [], "done_g": [], "oc_on": False,
                  "oc_next": 0, "gidx": {}}

            def repack(r0, r1, pool_ok=True):
                # block repacks via strided sbuf->sbuf DMA, by row range
                for b in range(2):
                    dma(qb[:, b * NQ + r0 * QW:b * NQ + r1 * QW],
                        qkvT_v[0:4, r0:r1, QW * b:QW * b + QW], pool_ok)
                    dma(kb[:, b * NK + r0 * KW:b * NK + r1 * KW],
                        qkvT_v[4:8, r0:r1, 16 * b:16 * b + KW], pool_ok)
                    dma(vTb[:, b * NK + r0 * KW:b * NK + r1 * KW],
                        qkvT_v[8:12, r0:r1, 16 * b:16 * b + KW], pool_ok)

            def emit_logits_exp(u):
                b, g, kt = u
                q0 = b * NQ + g * G
                lg = lgp.tile([128, G], f32, tag="lg")
                kap = kb[:, b * NK + 128 * kt:b * NK + 128 * (kt + 1)]
                for j in range(G // 512):
                    nc.tensor.matmul(
                        lg[:, 512 * j:512 * (j + 1)],
                        kap,
                        qb[:, q0 + 512 * j:q0 + 512 * (j + 1)],
                        start=True, stop=True,
                    )
                ex = exp_pool.tile([128, G], bf16, tag="ex")
                st["exs"][u] = ex
                st["elist"].append(u)
                if (kt, (g + b) % 2) in ACT_SET:
                    nc.scalar.activation(
                        ex[:], lg[:], mybir.ActivationFunctionType.Exp,
                        bias=zbias[:],
                    )
                else:
                    nc.vector.tensor_scalar(
                        ex[:].bitcast(i16), lg[:], A_EXP, B_EXP,
                        mybir.AluOpType.mult, mybir.AluOpType.add)

            def emit_av(u):
                # all 6 granules' accumulators live in disjoint 256B regions
                # of ONE psum bank: a single start/stop accumulation group
                # spans the whole attention, so granule g+1's AVs never wait
                # for granule g's readout
                b, g, kt = u
                if (b, g) not in st["gidx"]:
                    st["gidx"][(b, g)] = len(st["gidx"])
                gi = st["gidx"][(b, g)]
                av = st["avbig"][:, 64 * gi:64 * gi + 64]
                ex = st["exs"][u]
                vbase = (b * NKT + kt) * 8
                nav = st["avn"].get((b, g), 0)
                for qt in range(NQT):
                    nc.tensor.matmul(
                        av[:, 8 * qt:8 * qt + 8],
                        ex[:, 128 * qt:128 * (qt + 1)],
                        vp[:, vbase:vbase + 8],
                        start=(gi == 0 and nav == 0 and qt == 0),
                        stop=(gi == 5 and nav == NKT - 1 and qt == NQT - 1),
                        skip_group_check=True,
                    )
                st["avn"][(b, g)] = nav + 1
                if nav == NKT - 1:
                    epi1(b, g)
                    st["epi2q"].append([2, b, g])
                st["exs"].pop(u)

            def epi1(b, g):
                # normalize o = num/den on DVE(recip)+Pool(muls)
                gi = st["gidx"][(b, g)]
                av = st["avbig"][:, 64 * gi:64 * gi + 64]
                nc.vector.tensor_copy(av_sb[:], av[:])
                av_v = av_sb[:].rearrange("p (q e) -> p q e", e=8)
                nc.vector.reciprocal(rec[:], av_v[:, :, 4])
                for qt in range(NQT):
                    nc.gpsimd.tensor_scalar(
                        o_sb[:, DH * qt:DH * (qt + 1)],
                        av_sb[:, 8 * qt:8 * qt + DH],
                        rec[:, qt:qt + 1], None,
                        mybir.AluOpType.mult)

            def epi2(b, g):
                # transpose [128q,4] -> [4,128q] into a bitcast view of a
                # spare lg psum slot, stage to o^T, fire scatter waves
                q0 = b * NQ + g * G
                tps_f32 = lgp.tile([128, G], f32, tag="lg")
                tps = tps_f32[0:DH, 0:G // 2].bitcast(bf16)
                for qt in range(NQT):
                    nc.tensor.transpose(
                        tps[:, 128 * qt:128 * (qt + 1)],
                        o_sb[:, DH * qt:DH * (qt + 1)],
                        id128[:],
                    )
                nc.scalar.copy(oT[:, q0:q0 + G], tps[:])
                st["done_g"].append((b, g))
                if (b, 0) in st["done_g"] and (b, 1) in st["done_g"] \
                        and g == 1:
                    scatter(b, 0, 80)
                    if b == 1:
                        st["oc_on"] = True
                elif g == 2:
                    scatter(b, 80, H, pool_ok=(b == 0))

            def scatter(b, row0, row1, pool_ok=True):
                # write o^T rows [row0,row1) into the 9 tap-shifted
                # row-blocks of oo (sbuf->sbuf DMAs, row-aligned)
                oTb_v = oT[:, b * NQ:(b + 1) * NQ].rearrange(
                    "p (r c) -> p r c", c=QW)
                for t in range(9):
                    dh, dw = t // 3, t % 3
                    off = (GUARD + (1 - dh) * WP + (QW * b + 1 - dw)
                           + row0 * WP)
                    dst = oo[4 * t:4 * t + 4, off:off + (row1 - row0) * WP]
                    dst_v = dst.rearrange("p (r c) -> p r c", c=WP)
                    dma(dst_v[:, :, 0:QW], oTb_v[:, row0:row1], pool_ok)

            def advance(u):
                # one stream step: logits+exp for unit u; AV trails the
                # emission list by AV_LAG; epilogue-2 two steps later yet
                emit_logits_exp(u)
                while st["navd"] < len(st["elist"]) - AV_LAG:
                    emit_av(st["elist"][st["navd"]])
                    st["navd"] += 1
                for e in st["epi2q"]:
                    e[0] -= 1
                while st["epi2q"] and st["epi2q"][0][0] <= 0:
                    _, b2, g2 = st["epi2q"].pop(0)
                    epi2(b2, g2)
                if st["oc_on"] and st["oc_next"] < 9 \
                        and len(st["elist"]) % 4 == 0:
                    outconv_chunk(st["oc_next"], ops, "opsA")
                    st["oc_next"] += 1

            # ---- q/k/v conv: 3x3, tap pairs (dh,0)+(dh,1) packed on 128
            # partitions (xx rows 64:128 are +1 col shifted) + (dh,2)
            # singles; bias added in the DVE psum->SBUF staging copy.
            # Repacks go in 3 row-waves; once rows 0:48 are out (chunk 5),
            # attention units needing only those rows interleave with the
            # remaining chunks, so the exp engines start ~12us in.
            # 27 units fit the conv interleave slots (2/chunk for chunks
            # 6..8, 3/chunk for 9..15); all stay within repacked row waves
            conv_units = ([(0, 0, kt) for kt in range(12)]
                          + [(1, 0, kt) for kt in range(6)]
                          + [(0, 0, kt) for kt in range(12, 18)]
                          + [(1, 0, kt) for kt in range(6, 9)])
            assert len(conv_units) == 27
            with tc.tile_pool(name="cps", bufs=2, space="PSUM") as cps:
                for ci in range(NCHUNK):
                    ps = cps.tile([12, CN], f32, tag="cps")
                    f0 = ci * CHUNK_ROWS * WP
                    for dh in range(3):
                        s = f0 + dh * WP
                        nc.tensor.matmul(
                            ps[:], wqkv[:, 12 * dh:12 * (dh + 1)],
                            xx[:, s:s + CN],
                            start=(dh == 0), stop=False,
                        )
                        nc.tensor.matmul(
                            ps[:], wqkv[0:CIN, 36 + 12 * dh:36 + 12 * (dh + 1)],
                            xx[0:CIN, s + 2:s + 2 + CN],
                            start=False, stop=(dh == 2),
                        )
                    psv = ps[:].rearrange("p (r c) -> p r c", c=WP)
                    nc.vector.tensor_scalar_add(
                        qkvT[:, ci * CHUNK_ROWS * W:(ci + 1) * CHUNK_ROWS * W],
                        psv[:, :, 0:W], bias12[:])
                    if ci == 5:
                        repack(0, 48, pool_ok=False)
                    elif ci == 11:
                        repack(48, 96)
                    elif ci == 15:
                        repack(96, H)
                    if ci >= 6:
                        n0 = 2 * (ci - 6) if ci < 9 else 3 * (ci - 9) + 6
                        n1 = n0 + (2 if ci < 9 else 3)
                        for u in conv_units[n0:n1]:
                            emit_logits_exp(u)

            # ---- V' build: per-kt PE transpose of v^T [4,128] -> [128,4],
            # one strided DVE copy per block into vp (ones in cols 4:8
            # persist from the memset)
            with tc.tile_pool(name="vps", bufs=2, space="PSUM") as vps:
                for b in range(2):
                    vpp = vps.tile([128, NKT * DH], bf16, tag="vpp")
                    vpp_v = vpp[:].rearrange("p (t e) -> p t e", e=DH)
                    for k0, k1 in ((0, 12), (12, 24), (24, 32)):
                        for kt in range(k0, k1):
                            nc.tensor.transpose(
                                vpp[:, DH * kt:DH * (kt + 1)],
                                vTb[:, b * NK + 128 * kt:
                                    b * NK + 128 * (kt + 1)],
                                id4[:],
                            )
                        nc.vector.tensor_copy(
                            vp_v[:, b * NKT + k0:b * NKT + k1, 0:DH],
                            vpp_v[:, k0:k1])

            ops = stack.enter_context(
                tc.tile_pool(name="ops", bufs=1, space="PSUM"))
            outp_ap = outp_d.ap()
            oc_state = {"stage": None}

            def outconv_chunk(ci, pool, tag):
                ps = pool.tile([CIN, CN], f32, tag=tag)
                nc.tensor.matmul(
                    ps[:], wo36[:],
                    oo[:, GUARD + ci * CHUNK_ROWS * WP:
                       GUARD + ci * CHUNK_ROWS * WP + CN],
                    start=True, stop=True,
                )
                psv = ps[:].rearrange("p (r c) -> p r c", c=WP)
                if ci % 4 == 0:
                    oc_state["stage"] = ost.tile(
                        [CIN, 4 * CHUNK_ROWS * W], f32, tag="ost",
                        name=f"ostage_{ci}")
                stage = oc_state["stage"]
                sl = slice((ci % 4) * CHUNK_ROWS * W,
                           (ci % 4 + 1) * CHUNK_ROWS * W)
                if ci % 2 == 0:
                    nc.vector.tensor_copy(stage[:, sl], psv[:, :, 0:W])
                else:
                    nc.scalar.copy(stage[:, sl], psv[:, :, 0:W])
                if ci % 4 == 3:
                    dma(outp_ap[:, (ci - 3) * CHUNK_ROWS * W:
                                (ci + 1) * CHUNK_ROWS * W], stage[:],
                        pool_ok=(ci < 8))

            # ---- attention main stream; outconv chunks 0..7 interleave
            # with the last granule once both blocks' rows 0:80 are in oo.
            with tc.tile_pool(name="avs", bufs=1, space="PSUM") as avsp:
                st["avbig"] = avsp.tile([128, 6 * 64], f32, tag="av",
                                        name="av_big")
                done = set(conv_units)
                for (b, g) in SEQ:
                    for kt in range(NKT):
                        if (b, g, kt) not in done:
                            advance((b, g, kt))
                # drain: AVs, last epilogue
                while st["navd"] < len(st["elist"]):
                    emit_av(st["elist"][st["navd"]])
                    st["navd"] += 1
                while st["epi2q"]:
                    _, b2, g2 = st["epi2q"].pop(0)
                    epi2(b2, g2)

            # ---- output conv tail: remaining chunks, ping-pong between
            # the ops bank and the freed avs bank
            with tc.tile_pool(name="ops2", bufs=1, space="PSUM") as ops2:
                for ci in range(9, NCHUNK):
                    if ci % 2:
                        outconv_chunk(ci, ops2, "opsB")
                    else:
                        outconv_chunk(ci, ops, "opsA")

        stack.close()

    nc.compile()
    return nc


def ml_bf16():
    import ml_dtypes
    return ml_dtypes.bfloat16


def _prep_inputs(x, wq, bq, wk, bk, wv, bv, wo):
    f32 = np.float32
    x = np.ascontiguousarray(np.asarray(x, f32))
    scale = f32(DH) ** -0.5

    bf = ml_bf16()
    xx = np.zeros((128, PADN), np.float32)
    xv = xx[:CIN, :HP * WP].reshape(CIN, HP, WP)
    xv[:, 1:1 + H, 1:1 + W] = x[0].transpose(2, 0, 1)
    xx[CIN:, :PADN - 1] = xx[:CIN, 1:]
    xx = xx.astype(bf)

    wq = np.asarray(wq, f32) * scale
    bq = np.asarray(bq, f32) * scale
    wk = np.asarray(wk, f32)
    bk = np.asarray(bk, f32)
    wv = np.asarray(wv, f32)
    bv = np.asarray(bv, f32)
    wo = np.asarray(wo, f32)

    id4 = np.eye(DH, dtype=bf)
    id128 = np.eye(128, dtype=bf)
    in_maps = []
    for h in range(NH):
        sl = slice(4 * h, 4 * h + 4)
        wqkv = np.zeros((128, 6, 12), f32)
        for dh in range(3):
            for p, dw in ((0, 0), (1, 1)):   # pair slots on partition halves
                wqkv[64 * p:64 * p + CIN, dh, 0:4] = wq[dh, dw, :, sl]
                wqkv[64 * p:64 * p + CIN, dh, 4:8] = wk[dh, dw, :, sl]
                wqkv[64 * p:64 * p + CIN, dh, 8:12] = wv[dh, dw, :, sl]
            wqkv[:CIN, 3 + dh, 0:4] = wq[dh, 2, :, sl]
            wqkv[:CIN, 3 + dh, 4:8] = wk[dh, 2, :, sl]
            wqkv[:CIN, 3 + dh, 8:12] = wv[dh, 2, :, sl]
        bias12 = np.concatenate([bq[sl], bk[sl], bv[sl]]).reshape(12, 1)
        wo36 = np.zeros((36, 64), f32)
        for dh in range(3):
            for dw in range(3):
                wo36[(3 * dh + dw) * 4:(3 * dh + dw) * 4 + 4] = wo[dh, dw, sl, :]
        in_maps.append({
            "xx": xx,
            "bias12": np.ascontiguousarray(bias12.astype(f32)),
            "wqkv": np.ascontiguousarray(wqkv.reshape(128, 6 * 12).astype(bf)),
            "wo36": np.ascontiguousarray(wo36.astype(bf)),
            "id4": id4,
            "id128": id128,
        })
    return in_maps


def _run(in_maps, trace=False, trace_cores=None):
    from concourse.bass_utils import run_bass_kernel_spmd

    if "nc" not in _cached:
        _cached["nc"] = _build_nc()
    return run_bass_kernel_spmd(
        _cached["nc"], in_maps, core_ids=list(range(NH)),
        trace=trace, trace_cores=trace_cores,
    )


def kernel(x, wq, bq, wk, bk, wv, bv, wo):
    in_maps = _prep_inputs(x, wq, bq, wk, bk, wv, bv, wo)
    res = _run(in_maps)
    acc = np.zeros((CIN, NPIX), np.float64)
    for r in res.results:
        acc += r["outp"].astype(np.float64)
    out = acc.astype(np.float32).reshape(CIN, H, W).transpose(1, 2, 0)
    return out[None]
